# revision 46
# baseline (speedup 1.0000x reference)
"""Multi-head attention (B=2, S=2048, E=1024, H=16, D=64) on 8 TRN2 NeuronCores.

Sharding: data parallel over batch (2) x head-group parallel (4 groups of 4
heads). Each core computes Q/K/V projections for its 4 heads, full-sequence
attention for those heads, and a partial output projection (its heads' rows of
Wo). The host sums the 4 bf16 partial outputs per batch and adds the bias.

Design (v2, rebuilt around the ScalarE exp bottleneck = 128 x ~1.26us):
  - x streams in as 4 s-quarters on the ACT HWDGE ring; weights on the SP
    ring. K/Q/V projections are produced just-in-time so the first exp fires
    ~8us in and the kc-loop never starves on DMA.
  - PSUM: psL ping/pong (4 banks), psO 1 bank (both row-quadrants accumulate
    into the same region), psS 1 bank (same trick), 2 misc banks for
    projection / out-projection drains.
  - exp destination is contiguous per call ([P, kc, 2h, q] layout).
  - Softmax-denominator: DVE pairwise tree (16->8) + all-ones matmuls, run
    eagerly in slots 10..17 of the same iteration (no deferred tail).
  - Output projection drains as 1-bank m-units; output is bf16, batched into
    one DMA per q-chunk. Host sums partials in f32.
  - PE warmup matmuls at t=0 keep HAM at full clock when real work lands.
"""

import numpy as np
import ml_dtypes

import concourse.bass as bass
import concourse.mybir as mybir
import concourse.tile as tile
from concourse import bacc
from concourse import bass_utils
from contextlib import ExitStack

P = 128
B, S, E = 2, 2048, 1024
H, D = 16, 64
NCORES = 8
GROUPS = NCORES // B          # 4 head-groups per batch
HPG = H // GROUPS             # 4 heads per core
DHG = HPG * D                 # 256 head dims per core
NHP = HPG // 2                # 2 head-pairs per core
EC = E // P                   # 8 e-chunks of 128
KC = S // P                   # 16 key chunks of 128
QCW = 512                     # q-chunk width
NQC = S // QCW                # 4 q chunks
SCALE = float(D) ** -0.5

BF16 = mybir.dt.bfloat16
F32 = mybir.dt.float32
EXP = mybir.ActivationFunctionType.Exp

# Schraudolph exp-from-bits: exp(SCALE*l) ~ bf16_frombits(round(A*l + B)),
# max rel err ~3.3%; numerator and denominator share the error so most of it
# cancels in the softmax. Used on a few key-chunks per iteration (VectorE).
SCH_A = 23.083120654223414          # 128 * SCALE * log2(e)
SCH_B = 16250.65                    # 128*127 - C, C tuned for minimax rel err
SCHRAUD_KC = (1, 3, 5, 7)

_NC = None


def _emit(tc):
    nc = tc.nc
    # all inputs pre-permuted on the host into SBUF layout -> every DMA is a
    # single contiguous run per partition (max descriptor efficiency)
    xq = nc.dram_tensor("xq", [NQC, P, EC, QCW], BF16, kind="ExternalInput").ap()
    wqL = nc.dram_tensor("wqL", [P, EC, DHG], BF16, kind="ExternalInput").ap()
    wkL = nc.dram_tensor("wkL", [P, EC, DHG], BF16, kind="ExternalInput").ap()
    wvL = nc.dram_tensor("wvL", [P, EC, DHG], BF16, kind="ExternalInput").ap()
    woL = nc.dram_tensor("woL", [P, NHP, E], BF16, kind="ExternalInput").ap()
    outT = nc.dram_tensor("outT", [E, S], BF16, kind="ExternalOutput").ap()

    mm = nc.tensor.matmul

    with ExitStack() as ctx:
        consts = ctx.enter_context(tc.tile_pool(name="consts", bufs=1))
        xp = ctx.enter_context(tc.tile_pool(name="xp", bufs=1))
        qkvp = ctx.enter_context(tc.tile_pool(name="qkvp", bufs=1))
        expp = ctx.enter_context(tc.tile_pool(name="expp", bufs=2))
        s1p = ctx.enter_context(tc.tile_pool(name="s1p", bufs=1))
        otnp = ctx.enter_context(tc.tile_pool(name="otnp", bufs=8))
        smallp = ctx.enter_context(tc.tile_pool(name="smallp", bufs=2))
        outfp = ctx.enter_context(tc.tile_pool(name="outfp", bufs=2))
        psumL = ctx.enter_context(tc.tile_pool(name="psumL", bufs=2, space="PSUM"))
        psumO = ctx.enter_context(tc.tile_pool(name="psumO", bufs=1, space="PSUM"))
        psumS = ctx.enter_context(tc.tile_pool(name="psumS", bufs=1, space="PSUM"))
        psumM = ctx.enter_context(tc.tile_pool(name="psumM", bufs=2, space="PSUM"))

        # ---- constants / warmup fodder ----
        ones = consts.tile([P, 64], BF16)
        zeros = consts.tile([P, 64], BF16)
        warm_s = consts.tile([P, P], BF16)
        warm_m = consts.tile([P, QCW], BF16)
        dum_i = consts.tile([P, 8], F32)
        dum_o = consts.tile([P, 8], BF16)
        nc.vector.memset(ones, 1.0)
        nc.vector.memset(zeros, 0.0)
        nc.vector.memset(warm_s, 0.01)
        nc.vector.memset(warm_m, 0.01)
        nc.vector.memset(dum_i, 0.0)

        # ---- input DMAs: x quarters on the ACT HWDGE ring, weights on SP ----
        x_sb = xp.tile([P, NQC, EC, QCW], BF16)   # quarter-major: contiguous dst
        wq_sb = consts.tile([P, EC, DHG], BF16)
        wk_sb = consts.tile([P, EC, DHG], BF16)
        wv_sb = consts.tile([P, EC, DHG], BF16)
        wo_sb = consts.tile([P, NHP, E], BF16)
        # single HWDGE ring, strict need-order: two parallel rings would
        # round-robin and halve the rate of whichever transfer is critical
        nc.scalar.dma_start(x_sb[:, 0, 0:4, :], xq[0, :, 0:4, :])
        nc.scalar.dma_start(wk_sb, wkL)
        nc.scalar.dma_start(wq_sb, wqL)
        nc.scalar.dma_start(x_sb[:, 0, 4:8, :], xq[0, :, 4:8, :])
        nc.scalar.dma_start(x_sb[:, 1, :, :], xq[1])
        nc.scalar.dma_start(wv_sb, wvL)
        nc.scalar.dma_start(x_sb[:, 2, :, :], xq[2])
        nc.scalar.dma_start(x_sb[:, 3, :, :], xq[3])
        nc.scalar.dma_start(wo_sb, woL)

        # exp table load off the critical path
        nc.scalar.activation(dum_o, dum_i, EXP)

        # PE warmup: keep HAM busy until real matmuls land (~4us of junk)
        psW = psumS.tile([P, QCW], F32, tag="psS", name="psW")
        for _ in range(6):
            mm(psW, lhsT=warm_s, rhs=warm_m, start=True, stop=True,
               skip_group_check=True)

        qt_sb = qkvp.tile([P, NHP, S], BF16)   # [dh-pair, hp, s]
        kt_sb = qkvp.tile([P, NHP, S], BF16)
        v_sb = qkvp.tile([P, KC, DHG], BF16)   # [s%128, s-chunk, dh]

        # ---- projection chains (plain 128x128 array mode) ----
        def v_chain(sc):
            ps = psumM.tile([P, QCW], F32, tag="m", name="psV")
            for ec in range(EC):
                mm(ps[:, 0:DHG],
                   lhsT=x_sb[:, sc // 4, ec, (sc % 4) * P:(sc % 4 + 1) * P],
                   rhs=wv_sb[:, ec, :],
                   start=(ec == 0), stop=(ec == EC - 1),
                   skip_group_check=True)
            nc.vector.tensor_copy(v_sb[:, sc, :], ps[:, 0:DHG])

        def qk_unit(w_sb, dst_sb, hp, scq, lo, hi, box):
            if lo == 0:
                box.append(psumM.tile([P, QCW], F32, tag="m", name="psQK"))
            ps = box[0]
            ssl = slice(scq * QCW, (scq + 1) * QCW)
            for ec in range(lo, hi):
                mm(ps,
                   lhsT=w_sb[:, ec, hp * P:(hp + 1) * P],
                   rhs=x_sb[:, scq, ec, :],
                   start=(ec == 0), stop=(ec == EC - 1),
                   skip_group_check=True)
            if hi == EC:
                nc.vector.tensor_copy(dst_sb[:, hp, ssl], ps)

        def qk_chain(w_sb, dst_sb, hp, scq):
            box = []
            qk_unit(w_sb, dst_sb, hp, scq, 0, EC, box)

        # ---- output projection (per q-chunk, 8 m-units + one bf16 DMA) ----
        otn_t = {}

        def op_unit(qc, m, box, phase=2, use_scalar=False):
            """phase 0: first contraction MM; phase 1: second MM + evac (+DMA
            at the half points); phase 2: both."""
            if m == 0 and phase != 1:
                box.append(outfp.tile([P, EC, QCW], BF16, tag="outf", name="outf"))
            if len(box) == 1:
                box.append(psumM.tile([P, QCW], F32, tag="m", name="psP"))
            of, ps = box[0], box[1]
            for hp2 in ((0, 1) if phase == 2 else (phase,)):
                mm(ps,
                   lhsT=wo_sb[:, hp2, m * P:(m + 1) * P],
                   rhs=otn_t[(hp2, qc)],
                   start=(hp2 == 0), stop=(hp2 == NHP - 1),
                   skip_group_check=True)
            if phase != 0:
                if use_scalar:
                    nc.scalar.copy(of[:, m, :], ps)
                else:
                    nc.vector.tensor_copy(of[:, m, :], ps)
                box.pop()
                if m == 3 or m == EC - 1:
                    half = slice(0, 4 * P) if m == 3 else slice(4 * P, EC * P)
                    nc.scalar.dma_start(
                        outT[half, qc * QCW:(qc + 1) * QCW].rearrange(
                            "(m p) q -> p m q", p=P),
                        of[:, 0 if m == 3 else 4:m + 1, :])

        # ---- deferred-work queues ----
        eager = []           # iter-1 V/K units, popped 2 per slot
        gen = []             # (cost_ns, fn) FIFO for Q chains + outproj units
        debt = [0.0]

        def pump(budget):
            """Drain deferred work; returns True if anything was emitted."""
            if eager:
                eager.pop(0)()
                if eager:
                    eager.pop(0)()
                return True
            debt[0] += budget
            did = False
            while gen and gen[0][0] <= debt[0]:
                cost, fn = gen.pop(0)
                fn()
                debt[0] -= cost
                did = True
            if not gen:
                debt[0] = 0.0
            return did

        # ---- softmax tail: normalize straight out of PSUM (no evac copies) ----
        def norm(hp, qc, psO, psS_):
            rec = smallp.tile([P, QCW], F32, tag="rec", name="rec")
            nc.vector.reciprocal_approx_fast(rec, psS_)
            otn = otnp.tile([P, QCW], BF16, tag="otn", name="otn")
            nc.vector.tensor_mul(otn, psO, rec)
            otn_t[(hp, qc)] = otn

        # tail of iteration (hp, qc): last attnV chunk (kc15) and the last two
        # denominator reductions, emitted in the NEXT iteration's slots 0/1 so
        # they queue behind the new logits and never block the first new exps
        def tail15(hp, qc, psO, psS_, exp_t, s1):
            c = KC - 1
            for h in range(2):
                col = hp * P + h * 64
                mm(psO[h * 64:(h + 1) * 64, :],
                   lhsT=v_sb[:, c, col:col + 64],
                   rhs=exp_t[:, c, h, :],
                   start=False, stop=True,
                   tile_position=(0, h * 64),
                   skip_group_check=True)

        def ones_j(psS_, s1, j):
            for h in range(2):
                mm(psS_[h * 64:(h + 1) * 64, :],
                   lhsT=ones[:, :],
                   rhs=s1[:, j, h, :],
                   start=(j == 0), stop=(j == 7),
                   tile_position=(0, h * 64),
                   skip_group_check=True)

        # ---- prologue: minimum work before the first exp ----
        kb, qb = [], []
        qk_unit(wk_sb, kt_sb, 0, 0, 0, 4, kb)  # K for kc 0-3 (ec halves so
        qk_unit(wq_sb, qt_sb, 0, 0, 0, 4, qb)  # each starts on its x half)
        qk_unit(wk_sb, kt_sb, 0, 0, 4, EC, kb)
        qk_unit(wq_sb, qt_sb, 0, 0, 4, EC, qb)
        v_chain(0)
        v_chain(1)

        # iter-1 eager schedule: interleave V chunks and K s-chunks so each
        # lands ahead of its consumer and behind its x-quarter's arrival.
        def qk_units(w_sb, dst_sb, hp, scq, step=1):
            box = []
            return [lambda lo=lo: qk_unit(w_sb, dst_sb, hp, scq, lo, lo + step, box)
                    for lo in range(0, EC, step)]

        k1 = qk_units(wk_sb, kt_sb, 0, 1, step=4)
        k2 = qk_units(wk_sb, kt_sb, 0, 2, step=4)
        k3 = qk_units(wk_sb, kt_sb, 0, 3, step=4)
        q01 = qk_units(wq_sb, qt_sb, 0, 1, step=4)
        eager.extend(k1)
        eager.extend([lambda: v_chain(2), lambda: v_chain(3),
                      lambda: v_chain(4), lambda: v_chain(5),
                      lambda: v_chain(6), lambda: v_chain(7)])
        eager.extend(k2)
        eager.extend([lambda: v_chain(8), lambda: v_chain(9),
                      lambda: v_chain(10), lambda: v_chain(11)])
        eager.extend(k3)
        eager.extend([lambda: v_chain(12), lambda: v_chain(13),
                      lambda: v_chain(14), lambda: v_chain(15)])
        eager.extend(q01)
        # remaining Q/K chains drain through the generic queue, in need order
        for w, dst, hp, scq in [
                (wq_sb, qt_sb, 0, 2), (wq_sb, qt_sb, 0, 3),
                (wk_sb, kt_sb, 1, 0), (wq_sb, qt_sb, 1, 0),
                (wk_sb, kt_sb, 1, 1), (wq_sb, qt_sb, 1, 1),
                (wk_sb, kt_sb, 1, 2), (wq_sb, qt_sb, 1, 2),
                (wk_sb, kt_sb, 1, 3), (wq_sb, qt_sb, 1, 3)]:
            for u in qk_units(w, dst, hp, scq):
                gen.append((280, u))

        # ---- main loop: 8 iterations of (head-pair, q-chunk) ----
        iters = [(hp, qc) for hp in range(NHP) for qc in range(NQC)]
        prev = None
        for it, (hp, qc) in enumerate(iters):
            qsl = slice(qc * QCW, (qc + 1) * QCW)
            exp_t = expp.tile([P, KC, 2, QCW], BF16, tag="exp", name="exp_t")
            psO = psumO.tile([P, QCW], F32, tag="psO", name="psO")
            psS_ = psumS.tile([P, QCW], F32, tag="psS", name="psS")
            s1 = s1p.tile([P, 8, 2, QCW], BF16, tag="s1", name="s1")
            for kcs in range(18):
                if kcs < KC:
                    c = kcs
                    psL = psumL.tile([P, 2, QCW], F32, tag="psL", name="psL")
                    for p2 in range(2):      # h inner: row-groups alternate so
                        for h in range(2):   # each LDWEIGHTS hides under the
                            hg = slice(h * 64, (h + 1) * 64)  # other row's MM
                            mm(psL[p2 * 64:(p2 + 1) * 64, h, :],
                               lhsT=kt_sb[hg, hp, c * P + p2 * 64:c * P + (p2 + 1) * 64],
                               rhs=qt_sb[hg, hp, qsl],
                               start=True, stop=True,
                               tile_position=(h * 64, p2 * 64),
                               skip_group_check=True)
                    if it >= 2 and c in SCHRAUD_KC:
                        # Schraudolph: exp(SCALE*l) ~ bf16_bits(A*l + B) on
                        # the idle VectorE, relieving the ScalarE bottleneck
                        nc.vector.tensor_scalar(
                            exp_t[:, c, :, :].bitcast(mybir.dt.int16), psL,
                            SCH_A, SCH_B,
                            mybir.AluOpType.mult, mybir.AluOpType.add)
                    else:
                        nc.scalar.activation(exp_t[:, c, :, :], psL, EXP,
                                             scale=SCALE)
                if kcs == 0 and prev is not None:
                    tail15(*prev)
                    ones_j(prev[3], prev[5], 6)
                if kcs == 1 and prev is not None:
                    ones_j(prev[3], prev[5], 7)
                if kcs == 2 and prev is not None:
                    norm(prev[0], prev[1], prev[2], prev[3])
                if kcs == 4 and prev is not None and prev[0] == 1:
                    box = []
                    for m in range(EC):
                        for phase in range(2):
                            gen.append((280, lambda q=prev[1], mi=m, ph=phase,
                                        b=box: op_unit(q, mi, b, phase=ph)))
                if 3 <= kcs:
                    c = kcs - 3
                    for h in range(2):
                        col = hp * P + h * 64
                        mm(psO[h * 64:(h + 1) * 64, :],
                           lhsT=v_sb[:, c, col:col + 64],
                           rhs=exp_t[:, c, h, :],
                           start=(c == 0), stop=False,
                           tile_position=(0, h * 64),
                           skip_group_check=True)
                if 9 <= kcs <= 16:
                    j = kcs - 9
                    nc.vector.tensor_add(
                        s1[:, j, :, :], exp_t[:, j, :, :], exp_t[:, j + 8, :, :])
                if 12 <= kcs:
                    ones_j(psS_, s1, kcs - 12)
                pump(0.0 if (it == 0 or kcs < 2) else 650.0)
            prev = (hp, qc, psO, psS_, exp_t, s1)

        # ---- epilogue: last iteration's tail + final out-projection ----
        ph, pq = prev[0], prev[1]
        tail15(*prev)
        ones_j(prev[3], prev[5], 6)
        ones_j(prev[3], prev[5], 7)
        norm(ph, pq, prev[2], prev[3])
        while gen or eager:
            pump(10 ** 9)
        box = []
        for m in range(EC):
            op_unit(pq, m, box, use_scalar=(m % 2 == 1))


def _build():
    nc = bacc.Bacc("TRN2", debug=False, target_bir_lowering=False)
    with tile.TileContext(nc) as tc:
        _emit(tc)
    nc.compile()
    return nc


def _get_nc():
    global _NC
    if _NC is None:
        _NC = _build()
    return _NC


def make_in_maps(x, Wq, Wk, Wv, Wo):
    bf = ml_dtypes.bfloat16
    x = np.asarray(x, np.float32)
    # xq[q, p, c, w] = x[b].T[c*128+p, q*512+w]
    xqb = [np.ascontiguousarray(
        x[b].T.reshape(EC, P, NQC, QCW).transpose(2, 1, 0, 3)).astype(bf)
        for b in range(B)]
    WqT = np.asarray(Wq, np.float32).T
    WkT = np.asarray(Wk, np.float32).T
    WvT = np.asarray(Wv, np.float32).T
    WoT = np.asarray(Wo, np.float32).T

    def wl(WT, sl):   # [E, DHG-slice] -> [P, EC, DHG]
        return np.ascontiguousarray(
            WT[:, sl].reshape(EC, P, DHG).transpose(1, 0, 2)).astype(bf)

    in_maps = []
    for c in range(NCORES):
        b, hg = divmod(c, GROUPS)
        sl = slice(hg * DHG, (hg + 1) * DHG)
        in_maps.append({
            "xq": xqb[b],
            "wqL": wl(WqT, sl),
            "wkL": wl(WkT, sl),
            "wvL": wl(WvT, sl),
            "woL": np.ascontiguousarray(
                WoT[sl, :].reshape(NHP, P, E).transpose(1, 0, 2)).astype(bf),
        })
    return in_maps


def run(in_maps, **kwargs):
    nc = _get_nc()
    return bass_utils.run_bass_kernel_spmd(
        nc, in_maps, core_ids=list(range(NCORES)), **kwargs)


def assemble(outs, bo):
    bo = np.asarray(bo, np.float32)
    out = np.empty((B, S, E), np.float32)
    for b in range(B):
        acc = outs[b * GROUPS]["outT"].astype(np.float32)
        for hg in range(1, GROUPS):
            acc += outs[b * GROUPS + hg]["outT"].astype(np.float32)
        out[b] = acc.T + bo
    return out


def kernel(x, Wq, Wk, Wv, Wo, bo):
    in_maps = make_in_maps(x, Wq, Wk, Wv, Wo)
    res = run(in_maps)
    return assemble(res.results, bo)
